# revision 31
# baseline (speedup 1.0000x reference)
"""BTV loss kernel for Trainium2 (8 NeuronCores, Bass/Tile) — v2.

reference: total = sum over 7x7 neighborhood shifts (k,l) != (0,0) of
           sqrt((x - roll(x,(k,l),axis=(2,3)))**2 + 1e-6).sum()
           out = 0.1 * total / x.size

Math:
  - circular-shift symmetry: shift (k,l) ~ (-k,-l); compute the 24
    half-space shifts {k>0, any l} u {k==0, l>0} and double.
  - sqrt(d^2 + 1e-6) ~= |d| (rel err ~3e-6); bf16 inputs add ~1e-5.

v2 schedule (all DVE ops use the "full" CROSS variant, zero waste):
  - DVE k0 op: in0 = x[., c], in1 = x[., c+2] (windowed, exact):
      F -> (0,2) all cols, E -> (0,3) even, O -> (0,1) odd.
  - DVE trio ops d in {-2,0,+2}: in0 = base rows bcast x3, in1 = t123
    (rows p+1..p+3), even deltas only so every stream stays 4B-aligned:
      per j: F -> (j,d) all, E -> (j,d+1) even, O -> (j,d-1) odd.
    Covers per j: l in {-2..2} fully + (j,-3) odd + (j,3) even.
  - PE+ACT: the 24 leftover strided half-slots: per j (j,-3) even and
    (j,3) odd; k0 (0,1) even and (0,3) odd; x 3 imgs. diff via +I/-I
    matmuls into PSUM, ACT Abs + accum_out.
  - t123 is built by SBUF->SBUF DMA from the te tiles (partition-shifted
    copies) instead of re-reading HBM: HBM traffic drops 5x to ~6.3MB
    per core (one bf16 pass).

Distribution: pure data parallel over the 24 (b,c) images, 3 per core;
host sums the 8 per-core partials in f64.
"""

import dataclasses
import re
from operator import add as _py_add

import numpy as np

import concourse.bass as bass
import concourse.bacc as bacc_mod
import concourse.mybir as mybir
from concourse import dve_ops as _dvo
from concourse.dve_spec import AluOp as _DveAluOp
from concourse.dve_spec import Bin, Spec, Src0, Src1
from concourse.tile import TileContext
from concourse.bass_utils import run_bass_kernel_spmd

from concourse.dve_uop import (
    ENABLE,
    AluInp,
    AluOp as UAluOp,
    DelayInp,
    InpSel,
    OutPath,
    OutSel,
    Trigger,
    UopConfig,
)

B, C, H, W = 8, 3, 1024, 1024
NCORES = 8
IMGS = (B * C) // NCORES        # images per core = 3
BASE = 4                        # left col pad (even => 4B-aligned in bf16)
WP = W + BASE + 3 + 1           # 1032: [w-4..w-1][0..1023][0,1,2][pad0]
RB = 128                        # rows per block (partition dim)
NBLK = H // RB                  # 8 row blocks per image
ROWS_BLK = RB + 3               # 131 rows stored per block (128 + 3 halo)
ROW = IMGS * WP                 # elements per stored row (3096)
T123W = 3 * ROW + 4             # t123 tile cols (head 2 + 3 rows + tail 2)

WEIGHT = 0.1
F32 = mybir.dt.float32
BF16 = mybir.dt.bfloat16

TRIO_D = (-2, 0, 2)             # trio deltas (even => aligned)
NGRP = 7                        # 28 PE slots / 4 per PSUM group
STAGE_COLS = 1 + NBLK * NGRP


def _mk_cross_uop(kind: str):
    """2x CROSS op ("full" variant). kind: "seed" | "steady".
    blocks: 0:d1  1:d4  2:d3  3:d2  4..6:sum tree  7:acc
    Per packed pair (a_lo,a_hi) vs (b_lo,b_hi):
      |a_lo-b_lo| + |a_hi-b_hi|  (shift d)
      |a_lo-b_hi| (shift d+1, even cols)   |a_hi-b_lo| (shift d-1, odd)
    """
    u = UopConfig()
    u.enable_input(InpSel.SRC_0, 0)      # a_lo -> ALU lane
    u.enable_input(InpSel.SRC_1, 1)      # b_lo -> delay lane 0
    u.enable_input(InpSel.SRC_0_HI, 2)   # a_hi -> delay lane 1
    u.enable_input(InpSel.SRC_1_HI, 3)   # b_hi -> delay lane 2
    u.accum_enabled = ENABLE
    dp = u.datapath_config
    dp[0].enable_alu(UAluOp.ABSOLUTE_DIFF, AluInp.PREV_ALU_OUT, AluInp.PREV_DELAY_0)
    dp[0].enable_delay_from_src(DelayInp.PREV_ALU_OUT, 3)
    dp[0].pass_through_delay(0, 1, 2)
    dp[1].enable_alu(UAluOp.ABSOLUTE_DIFF, AluInp.PREV_DELAY_1, AluInp.PREV_DELAY_0)
    dp[1].enable_delay_from_src(DelayInp.PREV_ALU_OUT, 0)
    dp[1].pass_through_delay(1, 2, 3)
    dp[2].enable_alu(UAluOp.ABSOLUTE_DIFF, AluInp.PREV_DELAY_3, AluInp.PREV_DELAY_2)
    dp[2].enable_delay_from_src(DelayInp.PREV_ALU_OUT, 3)
    dp[2].pass_through_delay(0, 1, 2)
    dp[3].enable_alu(UAluOp.ABSOLUTE_DIFF, AluInp.PREV_DELAY_1, AluInp.PREV_DELAY_2)
    dp[3].enable_delay_from_src(DelayInp.PREV_ALU_OUT, 1)
    dp[3].pass_through_delay(0, 3)
    dp[4].enable_alu(UAluOp.ADD, AluInp.PREV_ALU_OUT, AluInp.PREV_DELAY_1)
    dp[4].pass_through_delay(0, 3)
    dp[5].enable_alu(UAluOp.ADD, AluInp.PREV_ALU_OUT, AluInp.PREV_DELAY_0)
    dp[5].pass_through_delay(3)
    dp[6].enable_alu(UAluOp.ADD, AluInp.PREV_ALU_OUT, AluInp.PREV_DELAY_3)
    if kind == "seed":
        dp[7].enable_alu(UAluOp.BYPASS, AluInp.PREV_ALU_OUT, AluInp.PREV_ALU_OUT)
    else:
        dp[7].enable_alu(UAluOp.ADD, AluInp.CURR_ALU_OUT, AluInp.PREV_ALU_OUT)
    dp[7].alu_out_a_enable = ENABLE
    u.require_inp0 = ENABLE
    u.require_inp1 = ENABLE
    u.enable_output(OutSel.ALU_OUT, OutPath.WR0_LO)
    u.enable_output(OutSel.ALU_OUT, OutPath.WR0_HI)
    if kind == "seed":
        u.trigger = (Trigger.COUNT, Trigger.SRC_TENSOR_DONE, Trigger.NONE)
        u.next_uop = (1, 0, 0)
        u.repeat_count = 1
    else:
        u.trigger = (Trigger.SRC_TENSOR_DONE, Trigger.NONE, Trigger.NONE)
        u.next_uop = (0, 0, 0)
    return u


def _mk_poison_1x():
    """1x fallback: acc <- +inf so any non-2x execution is caught."""
    u = UopConfig()
    u.enable_input(InpSel.SRC_0, 0)
    u.enable_input(InpSel.POS_INF, 1)
    dp = u.datapath_config
    for b in range(7):
        dp[b].enable_alu(UAluOp.BYPASS, AluInp.PREV_ALU_OUT, AluInp.PREV_ALU_OUT)
        dp[b].pass_through_delay(0)
    dp[7].enable_alu(UAluOp.BYPASS, AluInp.PREV_DELAY_0, AluInp.PREV_DELAY_0)
    dp[7].alu_out_a_enable = ENABLE
    u.accum_enabled = ENABLE
    u.require_inp0 = ENABLE
    u.require_inp1 = ENABLE
    u.enable_output(OutSel.ALU_OUT, OutPath.WR0_LO)
    u.trigger = (Trigger.SRC_TENSOR_DONE, Trigger.NONE, Trigger.NONE)
    u.next_uop = (0, 0, 0)
    return u


def _mk_read_uop():
    """Route blk7's accumulator flop to the output (1-element stream)."""
    u = UopConfig()
    u.enable_input(InpSel.SRC_0, 0)
    dp = u.datapath_config
    for b in range(7):
        dp[b].enable_alu(UAluOp.BYPASS, AluInp.PREV_ALU_OUT, AluInp.PREV_ALU_OUT)
    dp[7].enable_alu(UAluOp.BYPASS, AluInp.CURR_ALU_OUT, AluInp.CURR_ALU_OUT)
    u.require_inp0 = ENABLE
    u.enable_output(OutSel.ALU_OUT, OutPath.WR0_LO)
    u.trigger = (Trigger.SRC_TENSOR_DONE, Trigger.NONE, Trigger.NONE)
    u.next_uop = (0, 0, 0)
    return u


class _HandDveOp(_dvo.DveOp):
    BUILDERS = {}  # name -> (build_1x_list, build_2x_list_or_None, rd1_en)

    def compile(self, ver):
        from concourse.dve_uop import DveOpSpec

        key = (self.name, ver)
        if (r := _dvo._COMPILE_CACHE.get(key)) is not None:
            return r
        b1, b2, rd1 = self.BUILDERS[self.name]
        result = DveOpSpec(
            name=self.name,
            opcode=_dvo.get_dve_sub_opcode(self.name),
            uops=b1(),
            uops_2x=(b2() if b2 is not None else None),
            rd1_en=rd1,
        )
        got = result.sha(ver)
        if self.uops_sha.get(ver) != got:
            raise ValueError(f"sha drift ({ver}: {got} != pinned)")
        _dvo._COMPILE_CACHE[key] = result
        return result


def _register(name, spec, build_1x, build_2x, rd1_en):
    _HandDveOp.BUILDERS[name] = (build_1x, build_2x, rd1_en)
    op = _HandDveOp(name, spec, subdim=False, uops_sha={})
    _dvo._SUB_OPCODE_FOR_NAME[name] = _dvo._CUSTOM_DVE_ROW_BASE + len(_dvo.OPS)
    shas = {}
    for ver in ("v3", "v4"):
        try:
            op.compile(ver)
            shas[ver] = op.uops_sha.get(ver)
        except ValueError as e:
            m = re.search(r"([0-9a-f]{16})", str(e))
            if not m:
                raise
            shas[ver] = m.group(1)
    op = dataclasses.replace(op, uops_sha=shas)
    _dvo.OPS.append(op)
    _dvo.CUSTOM_DVE_SPECS[name] = spec
    return op


_OPS = None


def _get_ops():
    """dict: 'seed' | 'cont' | 'read' -> op."""
    global _OPS
    if _OPS is not None:
        return _OPS
    have = {op.name: op for op in _dvo.OPS}
    names = {"seed": "XR_SEED_F_ANT", "cont": "XR_CONT_F_ANT"}
    if names["seed"] in have and "XR_READ_ANT" in have:
        _OPS = {k: have[n] for k, n in names.items()}
        _OPS["read"] = have["XR_READ_ANT"]
        return _OPS

    def _ref(in0, in1, s0, s1, imm2):
        a = in0.astype(np.float32)
        b = in1.astype(np.float32)
        P = a.shape[0]
        out = np.abs(a.reshape(P, -1) - b.reshape(P, -1))
        return out.reshape(in0.shape), out.reshape(P, -1).sum(-1, keepdims=True)

    spec_acc = Spec(
        body=Bin(_DveAluOp.ABSOLUTE_DIFF, Src0, Src1),
        accum=_py_add,
        reference=_ref,
    )
    spec_read = Spec(
        body=Src0,
        reference=lambda in0, in1, s0, s1, imm2: in0.astype(np.float32),
    )
    _OPS = {}
    for kind, name in names.items():
        _OPS[kind] = _register(
            name,
            spec_acc,
            lambda: [_mk_poison_1x(), _mk_poison_1x()],
            lambda kind=kind: [_mk_cross_uop(kind), _mk_cross_uop("steady")],
            True,
        )
    _OPS["read"] = _register(
        "XR_READ_ANT", spec_read, lambda: [_mk_read_uop()], None, False
    )
    return _OPS


def _pe_slots(te, t123):
    """The 24 strided 512-wide (base, shift) rhs pairs for one block."""
    slots = []
    for j in (1, 2, 3):
        sec = 2 + (j - 1) * ROW
        for i in range(IMGS):
            # (j,-3) even: a = col B+2c, b = row p+j col B-3+2c
            a = te[:, i, BASE:BASE + W]
            a = a.rearrange("p (c t) -> p c t", t=2)[:, :, 0]
            bcol = sec + i * WP + BASE - 3
            bs = t123[:, bcol:bcol + W]
            bs = bs.rearrange("p (c t) -> p c t", t=2)[:, :, 0]
            slots.append((a, bs))
            # (j,+3) odd: a = col B+1+2c, b = row p+j col B+4+2c
            a = te[:, i, BASE + 1:BASE + 1 + W]
            a = a.rearrange("p (c t) -> p c t", t=2)[:, :, 0]
            bcol = sec + i * WP + BASE + 4
            bs = t123[:, bcol:bcol + W]
            bs = bs.rearrange("p (c t) -> p c t", t=2)[:, :, 0]
            slots.append((a, bs))
    for i in range(IMGS):
        # k0 (0,1) even: a = col B+2c, b = col B+1+2c
        a = te[:, i, BASE:BASE + W]
        a = a.rearrange("p (c t) -> p c t", t=2)[:, :, 0]
        bs = te[:, i, BASE + 1:BASE + 1 + W]
        bs = bs.rearrange("p (c t) -> p c t", t=2)[:, :, 0]
        slots.append((a, bs))
        # k0 (0,3) odd: a = col B+1+2c, b = col B+4+2c
        a = te[:, i, BASE + 1:BASE + 1 + W]
        a = a.rearrange("p (c t) -> p c t", t=2)[:, :, 0]
        bs = te[:, i, BASE + 4:BASE + 4 + W]
        bs = bs.rearrange("p (c t) -> p c t", t=2)[:, :, 0]
        slots.append((a, bs))
    # img2's k0 coverage that the (2-img) DVE k0 op no longer provides:
    # (0,2) full width as two contiguous slots, (0,3) even, (0,1) odd
    for c0 in (0, 512):
        slots.append((
            te[:, 2, BASE + c0:BASE + c0 + 512],
            te[:, 2, BASE + 2 + c0:BASE + 2 + c0 + 512],
        ))
    a = te[:, 2, BASE:BASE + W]
    a = a.rearrange("p (c t) -> p c t", t=2)[:, :, 0]
    bs = te[:, 2, BASE + 3:BASE + 3 + W]
    bs = bs.rearrange("p (c t) -> p c t", t=2)[:, :, 0]
    slots.append((a, bs))
    a = te[:, 2, BASE + 1:BASE + 1 + W]
    a = a.rearrange("p (c t) -> p c t", t=2)[:, :, 0]
    bs = te[:, 2, BASE + 2:BASE + 2 + W]
    bs = bs.rearrange("p (c t) -> p c t", t=2)[:, :, 0]
    slots.append((a, bs))
    return slots


def _build_nc():
    ops = _get_ops()
    nc = bacc_mod.Bacc("TRN2", target_bir_lowering=False)
    # host layout: flat rows 0..H-1; row q = [img0|img1|img2] each WP wide
    # (circular col pads baked in). No row halo: t123 is built on-chip.
    X = nc.dram_tensor(
        "x", [NBLK * ROWS_BLK * ROW + 8], BF16, kind="ExternalInput"
    )
    WI = nc.dram_tensor("wi", [128, 128], BF16, kind="ExternalInput")
    WNI = nc.dram_tensor("wni", [128, 128], BF16, kind="ExternalInput")
    OUT = nc.dram_tensor("out", [128, STAGE_COLS], F32, kind="ExternalOutput")

    with TileContext(nc) as tc:
        with (
            tc.tile_pool(name="te", bufs=2) as te_pool,
            tc.tile_pool(name="t123", bufs=2) as t123_pool,
            tc.tile_pool(name="sc", bufs=1) as sc_pool,
            tc.tile_pool(name="acc", bufs=1) as acc_pool,
            tc.psum_pool(name="ps", bufs=2) as ps_pool,
        ):
            stage = acc_pool.tile([128, STAGE_COLS], F32)
            scratch = sc_pool.tile([128, 3 * ROW], BF16)
            ascr = acc_pool.tile([128, 2], BF16)
            wi = acc_pool.tile([128, 128], BF16)
            wni = acc_pool.tile([128, 128], BF16)
            for r in range(NBLK):
                te_prev = te_pool.tile([128, IMGS, WP], BF16, tag="te")
                t123 = t123_pool.tile([128, T123W], BF16, tag="t123")
                # Blocks 0-1 FIFO on the sync ring (full HBM rate for the
                # pipeline fill); later blocks' big prefetch via GPSIMD's
                # SWDGE so the busy ACT engine issues no DMA. Block 0's
                # t123 is split into per-j section loads so the j=1 trio
                # starts before sections 2-3 land.
                off = (r * ROWS_BLK + 1) * ROW - 2
                nc.sync.dma_start(
                    out=te_prev[:],
                    in_=bass.AP(X, r * ROWS_BLK * ROW,
                                [[ROW, 128], [1, ROW]]),
                )
                if r == 0:
                    nc.sync.dma_start(
                        out=t123[:, 0:ROW + 4],
                        in_=bass.AP(X, off, [[ROW, 128], [1, ROW + 4]]),
                    )
                    nc.sync.dma_start(
                        out=t123[:, ROW + 4:2 * ROW + 4],
                        in_=bass.AP(X, off + ROW + 4, [[ROW, 128], [1, ROW]]),
                    )
                    nc.sync.dma_start(
                        out=t123[:, 2 * ROW + 4:3 * ROW + 4],
                        in_=bass.AP(X, off + 2 * ROW + 4,
                                    [[ROW, 128], [1, ROW]]),
                    )
                    # small constant loads on the idle SWDGE queue so they
                    # don't wait behind the 4MB of block-0 loads
                    nc.gpsimd.dma_start(out=wi[:], in_=WI[:])
                    nc.gpsimd.dma_start(out=wni[:], in_=WNI[:])
                    # pre-load the ACT Abs table before any DVE critical
                    # section (the lazy table-load DMA deadlocks against
                    # critical branches)
                    nc.scalar.activation(
                        out=ascr[:, 0:2],
                        in_=wi[:, 0:2],
                        func=mybir.ActivationFunctionType.Abs,
                    )
                else:
                    eng_b = nc.sync if r == 1 else nc.gpsimd
                    eng_b.dma_start(
                        out=t123[:],
                        in_=bass.AP(X, off, [[ROW, 128], [1, T123W]]),
                    )
                prev_f = te_prev[:].rearrange("p a b -> p (a b)")

                # --- DVE chain: k0 (te only, runs during t123 load),
                # then the j-merged trios (block 0: per-j ops so each is
                # gated only on its own t123 section).
                kind = "seed" if r == 0 else "cont"
                nc.vector._custom_dve(
                    ops[kind],
                    out=scratch[:, 0: 2 * W],
                    in0=te_prev[:, 0:2, BASE:BASE + W],
                    in1=te_prev[:, 0:2, BASE + 2:BASE + 2 + W],
                ).ins.perf_max = 1
                if r == 0:
                    # j=1 ops gated on section 1 only; j=2,3 merged (their
                    # sections land while the j=1 ops run)
                    for d in TRIO_D:
                        nc.vector._custom_dve(
                            ops["cont"],
                            out=scratch[:, 0:ROW],
                            in0=prev_f,
                            in1=t123[:, d + 2:d + 2 + ROW],
                        ).ins.perf_max = 1
                    in0b2 = prev_f.rearrange("p (x c) -> p x c", x=1)
                    in0b2 = in0b2.broadcast_to((128, 2, ROW))
                    for d in TRIO_D:
                        m0 = d + 2 + ROW
                        in1b2 = t123[:, m0:m0 + 2 * ROW].rearrange(
                            "p (j c) -> p j c", j=2
                        )
                        nc.vector._custom_dve(
                            ops["cont"],
                            out=scratch[:, 0:2 * ROW],
                            in0=in0b2,
                            in1=in1b2,
                        ).ins.perf_max = 1
                else:
                    in0b = prev_f.rearrange("p (x c) -> p x c", x=1)
                    in0b = in0b.broadcast_to((128, 3, ROW))
                    for d in TRIO_D:
                        m0 = d + 2
                        in1b = t123[:, m0:m0 + 3 * ROW].rearrange(
                            "p (j c) -> p j c", j=3
                        )
                        nc.vector._custom_dve(
                            ops["cont"],
                            out=scratch[:],
                            in0=in0b,
                            in1=in1b,
                        ).ins.perf_max = 1

                # --- PE + ACT: 24 strided slots in 6 groups of 4
                MMW = 512
                slots = _pe_slots(te_prev, t123)
                for g in range(0, len(slots), 4):
                    grp = slots[g:g + 4]
                    psum = ps_pool.tile([128, 4 * MMW], F32, tag="ps")
                    for m, (brhs, srhs) in enumerate(grp):
                        nc.tensor.matmul(
                            out=psum[:, m * MMW:(m + 1) * MMW],
                            lhsT=wi[:],
                            rhs=brhs,
                            start=True,
                            stop=False,
                        )
                    for m, (brhs, srhs) in enumerate(grp):
                        nc.tensor.matmul(
                            out=psum[:, m * MMW:(m + 1) * MMW],
                            lhsT=wni[:],
                            rhs=srhs,
                            start=False,
                            stop=True,
                        )
                    col = 1 + r * NGRP + g // 4
                    nc.scalar.activation(
                        out=psum[:, 0: len(grp) * MMW],
                        in_=psum[:, 0: len(grp) * MMW],
                        func=mybir.ActivationFunctionType.Abs,
                        accum_out=stage[:, col:col + 1],
                    )
            # ship the ACT columns while the last trio still runs; only
            # col 0 (the DVE accumulator) waits for the read op
            nc.sync.dma_start(
                out=bass.AP(OUT, 1, [[STAGE_COLS, 128], [1, STAGE_COLS - 1]]),
                in_=stage[:, 1:STAGE_COLS],
            )
            nc.vector._custom_dve(
                ops["read"],
                out=stage[:, 0:1],
                in0=scratch[:, 0:1],
            )
            nc.sync.dma_start(
                out=bass.AP(OUT, 0, [[STAGE_COLS, 128], [1, 1]]),
                in_=stage[:, 0:1],
            )
    return nc


_NC = None


def _get_nc():
    global _NC
    if _NC is None:
        _NC = _build_nc()
        if not _NC.is_finalized():
            _NC.finalize()
    return _NC


def _prep_shards(x: np.ndarray):
    """bf16-cast, circular col pad, flatten rows into per-core layout."""
    imgs = np.ascontiguousarray(x.reshape(B * C, H, W), dtype=np.float32)

    def to_bf16(a32):
        b = a32.view(np.uint32)
        return ((b + 0x7FFF + ((b >> 16) & 1)) >> 16).astype(np.uint16)

    imgs_b = to_bf16(imgs)  # (24, H, W) uint16 view of bf16
    HPAD = H + 3
    even = np.zeros((B * C, HPAD, WP), dtype=np.uint16)
    even[:, :H, BASE:BASE + W] = imgs_b
    even[:, :H, :BASE] = imgs_b[:, :, W - BASE:]
    even[:, :H, BASE + W:BASE + W + 3] = imgs_b[:, :, :3]
    even[:, H:, :] = even[:, :3, :]

    I = np.eye(128, dtype=np.float32)
    wi = to_bf16(I)
    wni = to_bf16(-I)

    shards = even.reshape(NCORES, IMGS, HPAD, WP)
    out = []
    pcorr = []
    for n in range(NCORES):
        t = shards[n].transpose(1, 0, 2)  # (HPAD, IMGS, WP)
        blk = np.empty((NBLK, ROWS_BLK, IMGS, WP), dtype=np.uint16)
        for r in range(NBLK):
            blk[r] = t[r * RB: r * RB + ROWS_BLK]
        flat = np.concatenate([blk.reshape(-1), np.zeros(8, np.uint16)])
        out.append({"x": flat, "wi": wi, "wni": wni})
        # Exact correction for the trio ops' pad-column junk terms: the
        # 4 pad a-pairs per img per row contribute F+E+O terms with
        # b read at flat offset +j*ROW+d, exactly as the device t123
        # tile is laid out.
        af = (flat.astype(np.uint32) << 16).view(np.float32).astype(np.float64)
        rowbase = (
            (ROWS_BLK * np.arange(NBLK)[:, None] + np.arange(128)[None, :])
            * ROW
        )
        P = 0.0
        for j in (1, 2, 3):
            for d in TRIO_D:
                for i in range(IMGS):
                    for c in (i * WP + 0, i * WP + 2,
                              i * WP + BASE + W, i * WP + BASE + W + 2):
                        ai = rowbase + c
                        bi = ai + j * ROW + d
                        alo, ahi = af[ai], af[ai + 1]
                        blo, bhi = af[bi], af[bi + 1]
                        P += (np.abs(alo - blo) + np.abs(ahi - bhi)
                              + np.abs(alo - bhi) + np.abs(ahi - blo)).sum()
        pcorr.append(P)
    return out, pcorr


def _run(x: np.ndarray, trace: bool = False):
    import ml_dtypes

    nc = _get_nc()
    in_maps, pcorr = _prep_shards(x)
    in_maps = [
        {k: v.view(ml_dtypes.bfloat16) for k, v in m.items()} for m in in_maps
    ]
    res = run_bass_kernel_spmd(
        nc, in_maps, core_ids=list(range(NCORES)), trace=trace
    )
    total = 0.0
    for r, pc in zip(res.results, pcorr):
        total += r["out"].astype(np.float64).sum() - pc
    val = WEIGHT * 2.0 * total / float(B * C * H * W)
    return np.float32(val), res


def kernel(x: np.ndarray) -> np.ndarray:
    x = np.asarray(x, dtype=np.float32)
    val, _ = _run(x, trace=False)
    return val


# revision 32
# speedup vs baseline: 1.1541x; 1.1541x over previous
"""BTV loss kernel for Trainium2 (8 NeuronCores, Bass/Tile) — v2.

reference: total = sum over 7x7 neighborhood shifts (k,l) != (0,0) of
           sqrt((x - roll(x,(k,l),axis=(2,3)))**2 + 1e-6).sum()
           out = 0.1 * total / x.size

Math:
  - circular-shift symmetry: shift (k,l) ~ (-k,-l); compute the 24
    half-space shifts {k>0, any l} u {k==0, l>0} and double.
  - sqrt(d^2 + 1e-6) ~= |d| (rel err ~3e-6); bf16 inputs add ~1e-5.

v2 schedule (all DVE ops use the "full" CROSS variant, zero waste):
  - DVE k0 op: in0 = x[., c], in1 = x[., c+2] (windowed, exact):
      F -> (0,2) all cols, E -> (0,3) even, O -> (0,1) odd.
  - DVE trio ops d in {-2,0,+2}: in0 = base rows bcast x3, in1 = t123
    (rows p+1..p+3), even deltas only so every stream stays 4B-aligned:
      per j: F -> (j,d) all, E -> (j,d+1) even, O -> (j,d-1) odd.
    Covers per j: l in {-2..2} fully + (j,-3) odd + (j,3) even.
  - PE+ACT: the 24 leftover strided half-slots: per j (j,-3) even and
    (j,3) odd; k0 (0,1) even and (0,3) odd; x 3 imgs. diff via +I/-I
    matmuls into PSUM, ACT Abs + accum_out.
  - t123 is built by SBUF->SBUF DMA from the te tiles (partition-shifted
    copies) instead of re-reading HBM: HBM traffic drops 5x to ~6.3MB
    per core (one bf16 pass).

Distribution: pure data parallel over the 24 (b,c) images, 3 per core;
host sums the 8 per-core partials in f64.
"""

import dataclasses
import re
from operator import add as _py_add

import numpy as np

import concourse.bass as bass
import concourse.bacc as bacc_mod
import concourse.mybir as mybir
from concourse import dve_ops as _dvo
from concourse.dve_spec import AluOp as _DveAluOp
from concourse.dve_spec import Bin, Spec, Src0, Src1
from concourse.tile import TileContext
from concourse.bass_utils import run_bass_kernel_spmd

from concourse.dve_uop import (
    ENABLE,
    AluInp,
    AluOp as UAluOp,
    DelayInp,
    InpSel,
    OutPath,
    OutSel,
    Trigger,
    UopConfig,
)

B, C, H, W = 8, 3, 1024, 1024
NCORES = 8
IMGS = (B * C) // NCORES        # images per core = 3
BASE = 4                        # left col pad (even => 4B-aligned in bf16)
WP = W + BASE + 3 + 1           # 1032: [w-4..w-1][0..1023][0,1,2][pad0]
RB = 128                        # rows per block (partition dim)
NBLK = H // RB                  # 8 row blocks per image
ROWS_BLK = RB + 3               # 131 rows stored per block (128 + 3 halo)
ROW = IMGS * WP                 # elements per stored row (3096)
T123W = 3 * ROW + 4             # t123 tile cols (head 2 + 3 rows + tail 2)

WEIGHT = 0.1
F32 = mybir.dt.float32
BF16 = mybir.dt.bfloat16

TRIO_D = (-2, 0, 2)             # trio deltas (even => aligned)
NGRP = 7                        # 28 PE slots / 4 per PSUM group
STAGE_COLS = 1 + NBLK * NGRP


def _mk_cross_uop(kind: str):
    """2x CROSS op ("full" variant). kind: "seed" | "steady".
    blocks: 0:d1  1:d4  2:d3  3:d2  4..6:sum tree  7:acc
    Per packed pair (a_lo,a_hi) vs (b_lo,b_hi):
      |a_lo-b_lo| + |a_hi-b_hi|  (shift d)
      |a_lo-b_hi| (shift d+1, even cols)   |a_hi-b_lo| (shift d-1, odd)
    """
    u = UopConfig()
    u.enable_input(InpSel.SRC_0, 0)      # a_lo -> ALU lane
    u.enable_input(InpSel.SRC_1, 1)      # b_lo -> delay lane 0
    u.enable_input(InpSel.SRC_0_HI, 2)   # a_hi -> delay lane 1
    u.enable_input(InpSel.SRC_1_HI, 3)   # b_hi -> delay lane 2
    u.accum_enabled = ENABLE
    dp = u.datapath_config
    dp[0].enable_alu(UAluOp.ABSOLUTE_DIFF, AluInp.PREV_ALU_OUT, AluInp.PREV_DELAY_0)
    dp[0].enable_delay_from_src(DelayInp.PREV_ALU_OUT, 3)
    dp[0].pass_through_delay(0, 1, 2)
    dp[1].enable_alu(UAluOp.ABSOLUTE_DIFF, AluInp.PREV_DELAY_1, AluInp.PREV_DELAY_0)
    dp[1].enable_delay_from_src(DelayInp.PREV_ALU_OUT, 0)
    dp[1].pass_through_delay(1, 2, 3)
    dp[2].enable_alu(UAluOp.ABSOLUTE_DIFF, AluInp.PREV_DELAY_3, AluInp.PREV_DELAY_2)
    dp[2].enable_delay_from_src(DelayInp.PREV_ALU_OUT, 3)
    dp[2].pass_through_delay(0, 1, 2)
    dp[3].enable_alu(UAluOp.ABSOLUTE_DIFF, AluInp.PREV_DELAY_1, AluInp.PREV_DELAY_2)
    dp[3].enable_delay_from_src(DelayInp.PREV_ALU_OUT, 1)
    dp[3].pass_through_delay(0, 3)
    dp[4].enable_alu(UAluOp.ADD, AluInp.PREV_ALU_OUT, AluInp.PREV_DELAY_1)
    dp[4].pass_through_delay(0, 3)
    dp[5].enable_alu(UAluOp.ADD, AluInp.PREV_ALU_OUT, AluInp.PREV_DELAY_0)
    dp[5].pass_through_delay(3)
    dp[6].enable_alu(UAluOp.ADD, AluInp.PREV_ALU_OUT, AluInp.PREV_DELAY_3)
    if kind == "seed":
        dp[7].enable_alu(UAluOp.BYPASS, AluInp.PREV_ALU_OUT, AluInp.PREV_ALU_OUT)
    else:
        dp[7].enable_alu(UAluOp.ADD, AluInp.CURR_ALU_OUT, AluInp.PREV_ALU_OUT)
    dp[7].alu_out_a_enable = ENABLE
    u.require_inp0 = ENABLE
    u.require_inp1 = ENABLE
    u.enable_output(OutSel.ALU_OUT, OutPath.WR0_LO)
    u.enable_output(OutSel.ALU_OUT, OutPath.WR0_HI)
    if kind == "seed":
        u.trigger = (Trigger.COUNT, Trigger.SRC_TENSOR_DONE, Trigger.NONE)
        u.next_uop = (1, 0, 0)
        u.repeat_count = 1
    else:
        u.trigger = (Trigger.SRC_TENSOR_DONE, Trigger.NONE, Trigger.NONE)
        u.next_uop = (0, 0, 0)
    return u


def _mk_poison_1x():
    """1x fallback: acc <- +inf so any non-2x execution is caught."""
    u = UopConfig()
    u.enable_input(InpSel.SRC_0, 0)
    u.enable_input(InpSel.POS_INF, 1)
    dp = u.datapath_config
    for b in range(7):
        dp[b].enable_alu(UAluOp.BYPASS, AluInp.PREV_ALU_OUT, AluInp.PREV_ALU_OUT)
        dp[b].pass_through_delay(0)
    dp[7].enable_alu(UAluOp.BYPASS, AluInp.PREV_DELAY_0, AluInp.PREV_DELAY_0)
    dp[7].alu_out_a_enable = ENABLE
    u.accum_enabled = ENABLE
    u.require_inp0 = ENABLE
    u.require_inp1 = ENABLE
    u.enable_output(OutSel.ALU_OUT, OutPath.WR0_LO)
    u.trigger = (Trigger.SRC_TENSOR_DONE, Trigger.NONE, Trigger.NONE)
    u.next_uop = (0, 0, 0)
    return u


def _mk_read_uop():
    """Route blk7's accumulator flop to the output (1-element stream)."""
    u = UopConfig()
    u.enable_input(InpSel.SRC_0, 0)
    dp = u.datapath_config
    for b in range(7):
        dp[b].enable_alu(UAluOp.BYPASS, AluInp.PREV_ALU_OUT, AluInp.PREV_ALU_OUT)
    dp[7].enable_alu(UAluOp.BYPASS, AluInp.CURR_ALU_OUT, AluInp.CURR_ALU_OUT)
    u.require_inp0 = ENABLE
    u.enable_output(OutSel.ALU_OUT, OutPath.WR0_LO)
    u.trigger = (Trigger.SRC_TENSOR_DONE, Trigger.NONE, Trigger.NONE)
    u.next_uop = (0, 0, 0)
    return u


class _HandDveOp(_dvo.DveOp):
    BUILDERS = {}  # name -> (build_1x_list, build_2x_list_or_None, rd1_en)

    def compile(self, ver):
        from concourse.dve_uop import DveOpSpec

        key = (self.name, ver)
        if (r := _dvo._COMPILE_CACHE.get(key)) is not None:
            return r
        b1, b2, rd1 = self.BUILDERS[self.name]
        result = DveOpSpec(
            name=self.name,
            opcode=_dvo.get_dve_sub_opcode(self.name),
            uops=b1(),
            uops_2x=(b2() if b2 is not None else None),
            rd1_en=rd1,
        )
        got = result.sha(ver)
        if self.uops_sha.get(ver) != got:
            raise ValueError(f"sha drift ({ver}: {got} != pinned)")
        _dvo._COMPILE_CACHE[key] = result
        return result


def _register(name, spec, build_1x, build_2x, rd1_en):
    _HandDveOp.BUILDERS[name] = (build_1x, build_2x, rd1_en)
    op = _HandDveOp(name, spec, subdim=False, uops_sha={})
    _dvo._SUB_OPCODE_FOR_NAME[name] = _dvo._CUSTOM_DVE_ROW_BASE + len(_dvo.OPS)
    shas = {}
    for ver in ("v3", "v4"):
        try:
            op.compile(ver)
            shas[ver] = op.uops_sha.get(ver)
        except ValueError as e:
            m = re.search(r"([0-9a-f]{16})", str(e))
            if not m:
                raise
            shas[ver] = m.group(1)
    op = dataclasses.replace(op, uops_sha=shas)
    _dvo.OPS.append(op)
    _dvo.CUSTOM_DVE_SPECS[name] = spec
    return op


_OPS = None


def _get_ops():
    """dict: 'seed' | 'cont' | 'read' -> op."""
    global _OPS
    if _OPS is not None:
        return _OPS
    have = {op.name: op for op in _dvo.OPS}
    names = {"seed": "XR_SEED_F_ANT", "cont": "XR_CONT_F_ANT"}
    if names["seed"] in have and "XR_READ_ANT" in have:
        _OPS = {k: have[n] for k, n in names.items()}
        _OPS["read"] = have["XR_READ_ANT"]
        return _OPS

    def _ref(in0, in1, s0, s1, imm2):
        a = in0.astype(np.float32)
        b = in1.astype(np.float32)
        P = a.shape[0]
        out = np.abs(a.reshape(P, -1) - b.reshape(P, -1))
        return out.reshape(in0.shape), out.reshape(P, -1).sum(-1, keepdims=True)

    spec_acc = Spec(
        body=Bin(_DveAluOp.ABSOLUTE_DIFF, Src0, Src1),
        accum=_py_add,
        reference=_ref,
    )
    spec_read = Spec(
        body=Src0,
        reference=lambda in0, in1, s0, s1, imm2: in0.astype(np.float32),
    )
    _OPS = {}
    for kind, name in names.items():
        _OPS[kind] = _register(
            name,
            spec_acc,
            lambda: [_mk_poison_1x(), _mk_poison_1x()],
            lambda kind=kind: [_mk_cross_uop(kind), _mk_cross_uop("steady")],
            True,
        )
    _OPS["read"] = _register(
        "XR_READ_ANT", spec_read, lambda: [_mk_read_uop()], None, False
    )
    return _OPS


def _pe_slots(te, t123):
    """The 24 strided 512-wide (base, shift) rhs pairs for one block."""
    slots = []
    for j in (1, 2, 3):
        sec = 2 + (j - 1) * ROW
        for i in range(IMGS):
            # (j,-3) even: a = col B+2c, b = row p+j col B-3+2c
            a = te[:, i, BASE:BASE + W]
            a = a.rearrange("p (c t) -> p c t", t=2)[:, :, 0]
            bcol = sec + i * WP + BASE - 3
            bs = t123[:, bcol:bcol + W]
            bs = bs.rearrange("p (c t) -> p c t", t=2)[:, :, 0]
            slots.append((a, bs))
            # (j,+3) odd: a = col B+1+2c, b = row p+j col B+4+2c
            a = te[:, i, BASE + 1:BASE + 1 + W]
            a = a.rearrange("p (c t) -> p c t", t=2)[:, :, 0]
            bcol = sec + i * WP + BASE + 4
            bs = t123[:, bcol:bcol + W]
            bs = bs.rearrange("p (c t) -> p c t", t=2)[:, :, 0]
            slots.append((a, bs))
    for i in range(IMGS):
        # k0 (0,1) even: a = col B+2c, b = col B+1+2c
        a = te[:, i, BASE:BASE + W]
        a = a.rearrange("p (c t) -> p c t", t=2)[:, :, 0]
        bs = te[:, i, BASE + 1:BASE + 1 + W]
        bs = bs.rearrange("p (c t) -> p c t", t=2)[:, :, 0]
        slots.append((a, bs))
        # k0 (0,3) odd: a = col B+1+2c, b = col B+4+2c
        a = te[:, i, BASE + 1:BASE + 1 + W]
        a = a.rearrange("p (c t) -> p c t", t=2)[:, :, 0]
        bs = te[:, i, BASE + 4:BASE + 4 + W]
        bs = bs.rearrange("p (c t) -> p c t", t=2)[:, :, 0]
        slots.append((a, bs))
    # img2's k0 coverage that the (2-img) DVE k0 op no longer provides:
    # (0,2) full width as two contiguous slots, (0,3) even, (0,1) odd
    for c0 in (0, 512):
        slots.append((
            te[:, 2, BASE + c0:BASE + c0 + 512],
            te[:, 2, BASE + 2 + c0:BASE + 2 + c0 + 512],
        ))
    a = te[:, 2, BASE:BASE + W]
    a = a.rearrange("p (c t) -> p c t", t=2)[:, :, 0]
    bs = te[:, 2, BASE + 3:BASE + 3 + W]
    bs = bs.rearrange("p (c t) -> p c t", t=2)[:, :, 0]
    slots.append((a, bs))
    a = te[:, 2, BASE + 1:BASE + 1 + W]
    a = a.rearrange("p (c t) -> p c t", t=2)[:, :, 0]
    bs = te[:, 2, BASE + 2:BASE + 2 + W]
    bs = bs.rearrange("p (c t) -> p c t", t=2)[:, :, 0]
    slots.append((a, bs))
    return slots


def _build_nc():
    ops = _get_ops()
    nc = bacc_mod.Bacc("TRN2", target_bir_lowering=False)
    # host layout: flat rows 0..H-1; row q = [img0|img1|img2] each WP wide
    # (circular col pads baked in). No row halo: t123 is built on-chip.
    X = nc.dram_tensor(
        "x", [NBLK * ROWS_BLK * ROW + 8], BF16, kind="ExternalInput"
    )
    WI = nc.dram_tensor("wi", [128, 128], BF16, kind="ExternalInput")
    WNI = nc.dram_tensor("wni", [128, 128], BF16, kind="ExternalInput")
    OUT = nc.dram_tensor("out", [128, STAGE_COLS], F32, kind="ExternalOutput")

    with TileContext(nc) as tc:
        with (
            tc.tile_pool(name="te", bufs=2) as te_pool,
            tc.tile_pool(name="t123", bufs=2) as t123_pool,
            tc.tile_pool(name="sc", bufs=1) as sc_pool,
            tc.tile_pool(name="acc", bufs=1) as acc_pool,
            tc.psum_pool(name="ps", bufs=2) as ps_pool,
        ):
            stage = acc_pool.tile([128, STAGE_COLS], F32)
            scratch = sc_pool.tile([128, 3 * ROW], BF16)
            ascr = acc_pool.tile([128, 2], BF16)
            wi = acc_pool.tile([128, 128], BF16)
            wni = acc_pool.tile([128, 128], BF16)
            for r in range(NBLK):
                te_prev = te_pool.tile([128, IMGS, WP], BF16, tag="te")
                t123 = t123_pool.tile([128, T123W], BF16, tag="t123")
                # Blocks 0-1 FIFO on the sync ring (full HBM rate for the
                # pipeline fill); later blocks' big prefetch via GPSIMD's
                # SWDGE so the busy ACT engine issues no DMA. Block 0's
                # t123 is split into per-j section loads so the j=1 trio
                # starts before sections 2-3 land.
                off = (r * ROWS_BLK + 1) * ROW - 2
                # block 0's te goes out on the SWDGE ring: its first byte
                # beats the sync ring's doorbell latency, and section 1
                # leads the sync queue in parallel
                te_eng = nc.gpsimd if r == 0 else nc.sync
                te_eng.dma_start(
                    out=te_prev[:],
                    in_=bass.AP(X, r * ROWS_BLK * ROW,
                                [[ROW, 128], [1, ROW]]),
                )
                if r == 0:
                    nc.sync.dma_start(
                        out=t123[:, 0:ROW + 4],
                        in_=bass.AP(X, off, [[ROW, 128], [1, ROW + 4]]),
                    )
                    nc.sync.dma_start(
                        out=t123[:, ROW + 4:2 * ROW + 4],
                        in_=bass.AP(X, off + ROW + 4, [[ROW, 128], [1, ROW]]),
                    )
                    nc.sync.dma_start(
                        out=t123[:, 2 * ROW + 4:3 * ROW + 4],
                        in_=bass.AP(X, off + 2 * ROW + 4,
                                    [[ROW, 128], [1, ROW]]),
                    )
                    # small constant loads on the idle SWDGE queue so they
                    # don't wait behind the 4MB of block-0 loads
                    nc.gpsimd.dma_start(out=wi[:], in_=WI[:])
                    nc.gpsimd.dma_start(out=wni[:], in_=WNI[:])
                    # pre-load the ACT Abs table before any DVE critical
                    # section (the lazy table-load DMA deadlocks against
                    # critical branches)
                    nc.scalar.activation(
                        out=ascr[:, 0:2],
                        in_=wi[:, 0:2],
                        func=mybir.ActivationFunctionType.Abs,
                    )
                else:
                    eng_b = nc.sync if r == 1 else nc.gpsimd
                    eng_b.dma_start(
                        out=t123[:],
                        in_=bass.AP(X, off, [[ROW, 128], [1, T123W]]),
                    )
                prev_f = te_prev[:].rearrange("p a b -> p (a b)")

                # --- DVE chain: k0 (te only, runs during t123 load),
                # then the j-merged trios (block 0: per-j ops so each is
                # gated only on its own t123 section).
                kind = "seed" if r == 0 else "cont"
                nc.vector._custom_dve(
                    ops[kind],
                    out=scratch[:, 0: 2 * W],
                    in0=te_prev[:, 0:2, BASE:BASE + W],
                    in1=te_prev[:, 0:2, BASE + 2:BASE + 2 + W],
                ).ins.perf_max = 1
                if r == 0:
                    # j=1 ops gated on section 1 only; j=2,3 merged (their
                    # sections land while the j=1 ops run)
                    for d in TRIO_D:
                        nc.vector._custom_dve(
                            ops["cont"],
                            out=scratch[:, 0:ROW],
                            in0=prev_f,
                            in1=t123[:, d + 2:d + 2 + ROW],
                        ).ins.perf_max = 1
                    in0b2 = prev_f.rearrange("p (x c) -> p x c", x=1)
                    in0b2 = in0b2.broadcast_to((128, 2, ROW))
                    for d in TRIO_D:
                        m0 = d + 2 + ROW
                        in1b2 = t123[:, m0:m0 + 2 * ROW].rearrange(
                            "p (j c) -> p j c", j=2
                        )
                        nc.vector._custom_dve(
                            ops["cont"],
                            out=scratch[:, 0:2 * ROW],
                            in0=in0b2,
                            in1=in1b2,
                        ).ins.perf_max = 1
                else:
                    in0b = prev_f.rearrange("p (x c) -> p x c", x=1)
                    in0b = in0b.broadcast_to((128, 3, ROW))
                    for d in TRIO_D:
                        m0 = d + 2
                        in1b = t123[:, m0:m0 + 3 * ROW].rearrange(
                            "p (j c) -> p j c", j=3
                        )
                        nc.vector._custom_dve(
                            ops["cont"],
                            out=scratch[:],
                            in0=in0b,
                            in1=in1b,
                        ).ins.perf_max = 1

                # --- PE + ACT: 24 strided slots in 6 groups of 4
                MMW = 512
                slots = _pe_slots(te_prev, t123)
                for g in range(0, len(slots), 4):
                    grp = slots[g:g + 4]
                    psum = ps_pool.tile([128, 4 * MMW], F32, tag="ps")
                    for m, (brhs, srhs) in enumerate(grp):
                        nc.tensor.matmul(
                            out=psum[:, m * MMW:(m + 1) * MMW],
                            lhsT=wi[:],
                            rhs=brhs,
                            start=True,
                            stop=False,
                        )
                    for m, (brhs, srhs) in enumerate(grp):
                        nc.tensor.matmul(
                            out=psum[:, m * MMW:(m + 1) * MMW],
                            lhsT=wni[:],
                            rhs=srhs,
                            start=False,
                            stop=True,
                        )
                    col = 1 + r * NGRP + g // 4
                    nc.scalar.activation(
                        out=psum[:, 0: len(grp) * MMW],
                        in_=psum[:, 0: len(grp) * MMW],
                        func=mybir.ActivationFunctionType.Abs,
                        accum_out=stage[:, col:col + 1],
                    )
            # ship the ACT columns while the last trio still runs; only
            # col 0 (the DVE accumulator) waits for the read op
            nc.sync.dma_start(
                out=bass.AP(OUT, 1, [[STAGE_COLS, 128], [1, STAGE_COLS - 1]]),
                in_=stage[:, 1:STAGE_COLS],
            )
            nc.vector._custom_dve(
                ops["read"],
                out=stage[:, 0:1],
                in0=scratch[:, 0:1],
            )
            nc.sync.dma_start(
                out=bass.AP(OUT, 0, [[STAGE_COLS, 128], [1, 1]]),
                in_=stage[:, 0:1],
            )
    return nc


_NC = None


def _get_nc():
    global _NC
    if _NC is None:
        _NC = _build_nc()
        if not _NC.is_finalized():
            _NC.finalize()
    return _NC


def _prep_shards(x: np.ndarray):
    """bf16-cast, circular col pad, flatten rows into per-core layout."""
    imgs = np.ascontiguousarray(x.reshape(B * C, H, W), dtype=np.float32)

    def to_bf16(a32):
        b = a32.view(np.uint32)
        return ((b + 0x7FFF + ((b >> 16) & 1)) >> 16).astype(np.uint16)

    imgs_b = to_bf16(imgs)  # (24, H, W) uint16 view of bf16
    HPAD = H + 3
    even = np.zeros((B * C, HPAD, WP), dtype=np.uint16)
    even[:, :H, BASE:BASE + W] = imgs_b
    even[:, :H, :BASE] = imgs_b[:, :, W - BASE:]
    even[:, :H, BASE + W:BASE + W + 3] = imgs_b[:, :, :3]
    even[:, H:, :] = even[:, :3, :]

    I = np.eye(128, dtype=np.float32)
    wi = to_bf16(I)
    wni = to_bf16(-I)

    shards = even.reshape(NCORES, IMGS, HPAD, WP)
    out = []
    pcorr = []
    for n in range(NCORES):
        t = shards[n].transpose(1, 0, 2)  # (HPAD, IMGS, WP)
        blk = np.empty((NBLK, ROWS_BLK, IMGS, WP), dtype=np.uint16)
        for r in range(NBLK):
            blk[r] = t[r * RB: r * RB + ROWS_BLK]
        flat = np.concatenate([blk.reshape(-1), np.zeros(8, np.uint16)])
        out.append({"x": flat, "wi": wi, "wni": wni})
        # Exact correction for the trio ops' pad-column junk terms: the
        # 4 pad a-pairs per img per row contribute F+E+O terms with
        # b read at flat offset +j*ROW+d, exactly as the device t123
        # tile is laid out.
        af = (flat.astype(np.uint32) << 16).view(np.float32).astype(np.float64)
        rowbase = (
            (ROWS_BLK * np.arange(NBLK)[:, None] + np.arange(128)[None, :])
            * ROW
        )
        P = 0.0
        for j in (1, 2, 3):
            for d in TRIO_D:
                for i in range(IMGS):
                    for c in (i * WP + 0, i * WP + 2,
                              i * WP + BASE + W, i * WP + BASE + W + 2):
                        ai = rowbase + c
                        bi = ai + j * ROW + d
                        alo, ahi = af[ai], af[ai + 1]
                        blo, bhi = af[bi], af[bi + 1]
                        P += (np.abs(alo - blo) + np.abs(ahi - bhi)
                              + np.abs(alo - bhi) + np.abs(ahi - blo)).sum()
        pcorr.append(P)
    return out, pcorr


def _run(x: np.ndarray, trace: bool = False):
    import ml_dtypes

    nc = _get_nc()
    in_maps, pcorr = _prep_shards(x)
    in_maps = [
        {k: v.view(ml_dtypes.bfloat16) for k, v in m.items()} for m in in_maps
    ]
    res = run_bass_kernel_spmd(
        nc, in_maps, core_ids=list(range(NCORES)), trace=trace
    )
    total = 0.0
    for r, pc in zip(res.results, pcorr):
        total += r["out"].astype(np.float64).sum() - pc
    val = WEIGHT * 2.0 * total / float(B * C * H * W)
    return np.float32(val), res


def kernel(x: np.ndarray) -> np.ndarray:
    x = np.asarray(x, dtype=np.float32)
    val, _ = _run(x, trace=False)
    return val


# revision 33
# speedup vs baseline: 1.1831x; 1.0251x over previous
"""BTV loss kernel for Trainium2 (8 NeuronCores, Bass/Tile) — v2.

reference: total = sum over 7x7 neighborhood shifts (k,l) != (0,0) of
           sqrt((x - roll(x,(k,l),axis=(2,3)))**2 + 1e-6).sum()
           out = 0.1 * total / x.size

Math:
  - circular-shift symmetry: shift (k,l) ~ (-k,-l); compute the 24
    half-space shifts {k>0, any l} u {k==0, l>0} and double.
  - sqrt(d^2 + 1e-6) ~= |d| (rel err ~3e-6); bf16 inputs add ~1e-5.

v2 schedule (all DVE ops use the "full" CROSS variant, zero waste):
  - DVE k0 op: in0 = x[., c], in1 = x[., c+2] (windowed, exact):
      F -> (0,2) all cols, E -> (0,3) even, O -> (0,1) odd.
  - DVE trio ops d in {-2,0,+2}: in0 = base rows bcast x3, in1 = t123
    (rows p+1..p+3), even deltas only so every stream stays 4B-aligned:
      per j: F -> (j,d) all, E -> (j,d+1) even, O -> (j,d-1) odd.
    Covers per j: l in {-2..2} fully + (j,-3) odd + (j,3) even.
  - PE+ACT: the 24 leftover strided half-slots: per j (j,-3) even and
    (j,3) odd; k0 (0,1) even and (0,3) odd; x 3 imgs. diff via +I/-I
    matmuls into PSUM, ACT Abs + accum_out.
  - t123 is built by SBUF->SBUF DMA from the te tiles (partition-shifted
    copies) instead of re-reading HBM: HBM traffic drops 5x to ~6.3MB
    per core (one bf16 pass).

Distribution: pure data parallel over the 24 (b,c) images, 3 per core;
host sums the 8 per-core partials in f64.
"""

import dataclasses
import re
from operator import add as _py_add

import numpy as np

import concourse.bass as bass
import concourse.bacc as bacc_mod
import concourse.mybir as mybir
from concourse import dve_ops as _dvo
from concourse.dve_spec import AluOp as _DveAluOp
from concourse.dve_spec import Bin, Spec, Src0, Src1
from concourse.tile import TileContext
from concourse.bass_utils import run_bass_kernel_spmd

from concourse.dve_uop import (
    ENABLE,
    AluInp,
    AluOp as UAluOp,
    DelayInp,
    InpSel,
    OutPath,
    OutSel,
    Trigger,
    UopConfig,
)

B, C, H, W = 8, 3, 1024, 1024
NCORES = 8
IMGS = (B * C) // NCORES        # images per core = 3
BASE = 4                        # left col pad (even => 4B-aligned in bf16)
WP = W + BASE + 3 + 1           # 1032: [w-4..w-1][0..1023][0,1,2][pad0]
RB = 128                        # rows per block (partition dim)
NBLK = H // RB                  # 8 row blocks per image
ROWS_BLK = RB + 3               # 131 rows stored per block (128 + 3 halo)
ROW = IMGS * WP                 # elements per stored row (3096)
T123W = 3 * ROW + 4             # t123 tile cols (head 2 + 3 rows + tail 2)

WEIGHT = 0.1
F32 = mybir.dt.float32
BF16 = mybir.dt.bfloat16

TRIO_D = (-2, 0, 2)             # trio deltas (even => aligned)
NGRP = 7                        # 28 PE slots / 4 per PSUM group
STAGE_COLS = 1 + NBLK * NGRP


def _mk_cross_uop(kind: str):
    """2x CROSS op ("full" variant). kind: "seed" | "steady".
    blocks: 0:d1  1:d4  2:d3  3:d2  4..6:sum tree  7:acc
    Per packed pair (a_lo,a_hi) vs (b_lo,b_hi):
      |a_lo-b_lo| + |a_hi-b_hi|  (shift d)
      |a_lo-b_hi| (shift d+1, even cols)   |a_hi-b_lo| (shift d-1, odd)
    """
    u = UopConfig()
    u.enable_input(InpSel.SRC_0, 0)      # a_lo -> ALU lane
    u.enable_input(InpSel.SRC_1, 1)      # b_lo -> delay lane 0
    u.enable_input(InpSel.SRC_0_HI, 2)   # a_hi -> delay lane 1
    u.enable_input(InpSel.SRC_1_HI, 3)   # b_hi -> delay lane 2
    u.accum_enabled = ENABLE
    dp = u.datapath_config
    dp[0].enable_alu(UAluOp.ABSOLUTE_DIFF, AluInp.PREV_ALU_OUT, AluInp.PREV_DELAY_0)
    dp[0].enable_delay_from_src(DelayInp.PREV_ALU_OUT, 3)
    dp[0].pass_through_delay(0, 1, 2)
    dp[1].enable_alu(UAluOp.ABSOLUTE_DIFF, AluInp.PREV_DELAY_1, AluInp.PREV_DELAY_0)
    dp[1].enable_delay_from_src(DelayInp.PREV_ALU_OUT, 0)
    dp[1].pass_through_delay(1, 2, 3)
    dp[2].enable_alu(UAluOp.ABSOLUTE_DIFF, AluInp.PREV_DELAY_3, AluInp.PREV_DELAY_2)
    dp[2].enable_delay_from_src(DelayInp.PREV_ALU_OUT, 3)
    dp[2].pass_through_delay(0, 1, 2)
    dp[3].enable_alu(UAluOp.ABSOLUTE_DIFF, AluInp.PREV_DELAY_1, AluInp.PREV_DELAY_2)
    dp[3].enable_delay_from_src(DelayInp.PREV_ALU_OUT, 1)
    dp[3].pass_through_delay(0, 3)
    dp[4].enable_alu(UAluOp.ADD, AluInp.PREV_ALU_OUT, AluInp.PREV_DELAY_1)
    dp[4].pass_through_delay(0, 3)
    dp[5].enable_alu(UAluOp.ADD, AluInp.PREV_ALU_OUT, AluInp.PREV_DELAY_0)
    dp[5].pass_through_delay(3)
    dp[6].enable_alu(UAluOp.ADD, AluInp.PREV_ALU_OUT, AluInp.PREV_DELAY_3)
    if kind == "seed":
        dp[7].enable_alu(UAluOp.BYPASS, AluInp.PREV_ALU_OUT, AluInp.PREV_ALU_OUT)
    else:
        dp[7].enable_alu(UAluOp.ADD, AluInp.CURR_ALU_OUT, AluInp.PREV_ALU_OUT)
    dp[7].alu_out_a_enable = ENABLE
    u.require_inp0 = ENABLE
    u.require_inp1 = ENABLE
    u.enable_output(OutSel.ALU_OUT, OutPath.WR0_LO)
    u.enable_output(OutSel.ALU_OUT, OutPath.WR0_HI)
    if kind == "seed":
        u.trigger = (Trigger.COUNT, Trigger.SRC_TENSOR_DONE, Trigger.NONE)
        u.next_uop = (1, 0, 0)
        u.repeat_count = 1
    else:
        u.trigger = (Trigger.SRC_TENSOR_DONE, Trigger.NONE, Trigger.NONE)
        u.next_uop = (0, 0, 0)
    return u


def _mk_poison_1x():
    """1x fallback: acc <- +inf so any non-2x execution is caught."""
    u = UopConfig()
    u.enable_input(InpSel.SRC_0, 0)
    u.enable_input(InpSel.POS_INF, 1)
    dp = u.datapath_config
    for b in range(7):
        dp[b].enable_alu(UAluOp.BYPASS, AluInp.PREV_ALU_OUT, AluInp.PREV_ALU_OUT)
        dp[b].pass_through_delay(0)
    dp[7].enable_alu(UAluOp.BYPASS, AluInp.PREV_DELAY_0, AluInp.PREV_DELAY_0)
    dp[7].alu_out_a_enable = ENABLE
    u.accum_enabled = ENABLE
    u.require_inp0 = ENABLE
    u.require_inp1 = ENABLE
    u.enable_output(OutSel.ALU_OUT, OutPath.WR0_LO)
    u.trigger = (Trigger.SRC_TENSOR_DONE, Trigger.NONE, Trigger.NONE)
    u.next_uop = (0, 0, 0)
    return u


def _mk_read_uop():
    """Route blk7's accumulator flop to the output (1-element stream)."""
    u = UopConfig()
    u.enable_input(InpSel.SRC_0, 0)
    dp = u.datapath_config
    for b in range(7):
        dp[b].enable_alu(UAluOp.BYPASS, AluInp.PREV_ALU_OUT, AluInp.PREV_ALU_OUT)
    dp[7].enable_alu(UAluOp.BYPASS, AluInp.CURR_ALU_OUT, AluInp.CURR_ALU_OUT)
    u.require_inp0 = ENABLE
    u.enable_output(OutSel.ALU_OUT, OutPath.WR0_LO)
    u.trigger = (Trigger.SRC_TENSOR_DONE, Trigger.NONE, Trigger.NONE)
    u.next_uop = (0, 0, 0)
    return u


class _HandDveOp(_dvo.DveOp):
    BUILDERS = {}  # name -> (build_1x_list, build_2x_list_or_None, rd1_en)

    def compile(self, ver):
        from concourse.dve_uop import DveOpSpec

        key = (self.name, ver)
        if (r := _dvo._COMPILE_CACHE.get(key)) is not None:
            return r
        b1, b2, rd1 = self.BUILDERS[self.name]
        result = DveOpSpec(
            name=self.name,
            opcode=_dvo.get_dve_sub_opcode(self.name),
            uops=b1(),
            uops_2x=(b2() if b2 is not None else None),
            rd1_en=rd1,
        )
        got = result.sha(ver)
        if self.uops_sha.get(ver) != got:
            raise ValueError(f"sha drift ({ver}: {got} != pinned)")
        _dvo._COMPILE_CACHE[key] = result
        return result


def _register(name, spec, build_1x, build_2x, rd1_en):
    _HandDveOp.BUILDERS[name] = (build_1x, build_2x, rd1_en)
    op = _HandDveOp(name, spec, subdim=False, uops_sha={})
    _dvo._SUB_OPCODE_FOR_NAME[name] = _dvo._CUSTOM_DVE_ROW_BASE + len(_dvo.OPS)
    shas = {}
    for ver in ("v3", "v4"):
        try:
            op.compile(ver)
            shas[ver] = op.uops_sha.get(ver)
        except ValueError as e:
            m = re.search(r"([0-9a-f]{16})", str(e))
            if not m:
                raise
            shas[ver] = m.group(1)
    op = dataclasses.replace(op, uops_sha=shas)
    _dvo.OPS.append(op)
    _dvo.CUSTOM_DVE_SPECS[name] = spec
    return op


_OPS = None


def _get_ops():
    """dict: 'seed' | 'cont' | 'read' -> op."""
    global _OPS
    if _OPS is not None:
        return _OPS
    have = {op.name: op for op in _dvo.OPS}
    names = {"seed": "XR_SEED_F_ANT", "cont": "XR_CONT_F_ANT"}
    if names["seed"] in have and "XR_READ_ANT" in have:
        _OPS = {k: have[n] for k, n in names.items()}
        _OPS["read"] = have["XR_READ_ANT"]
        return _OPS

    def _ref(in0, in1, s0, s1, imm2):
        a = in0.astype(np.float32)
        b = in1.astype(np.float32)
        P = a.shape[0]
        out = np.abs(a.reshape(P, -1) - b.reshape(P, -1))
        return out.reshape(in0.shape), out.reshape(P, -1).sum(-1, keepdims=True)

    spec_acc = Spec(
        body=Bin(_DveAluOp.ABSOLUTE_DIFF, Src0, Src1),
        accum=_py_add,
        reference=_ref,
    )
    spec_read = Spec(
        body=Src0,
        reference=lambda in0, in1, s0, s1, imm2: in0.astype(np.float32),
    )
    _OPS = {}
    for kind, name in names.items():
        _OPS[kind] = _register(
            name,
            spec_acc,
            lambda: [_mk_poison_1x(), _mk_poison_1x()],
            lambda kind=kind: [_mk_cross_uop(kind), _mk_cross_uop("steady")],
            True,
        )
    _OPS["read"] = _register(
        "XR_READ_ANT", spec_read, lambda: [_mk_read_uop()], None, False
    )
    return _OPS


def _pe_slots(te, t123):
    """The 24 strided 512-wide (base, shift) rhs pairs for one block."""
    slots = []
    for j in (1, 2, 3):
        sec = 2 + (j - 1) * ROW
        for i in range(IMGS):
            # (j,-3) even: a = col B+2c, b = row p+j col B-3+2c
            a = te[:, i, BASE:BASE + W]
            a = a.rearrange("p (c t) -> p c t", t=2)[:, :, 0]
            bcol = sec + i * WP + BASE - 3
            bs = t123[:, bcol:bcol + W]
            bs = bs.rearrange("p (c t) -> p c t", t=2)[:, :, 0]
            slots.append((a, bs))
            # (j,+3) odd: a = col B+1+2c, b = row p+j col B+4+2c
            a = te[:, i, BASE + 1:BASE + 1 + W]
            a = a.rearrange("p (c t) -> p c t", t=2)[:, :, 0]
            bcol = sec + i * WP + BASE + 4
            bs = t123[:, bcol:bcol + W]
            bs = bs.rearrange("p (c t) -> p c t", t=2)[:, :, 0]
            slots.append((a, bs))
    for i in range(IMGS):
        # k0 (0,1) even: a = col B+2c, b = col B+1+2c
        a = te[:, i, BASE:BASE + W]
        a = a.rearrange("p (c t) -> p c t", t=2)[:, :, 0]
        bs = te[:, i, BASE + 1:BASE + 1 + W]
        bs = bs.rearrange("p (c t) -> p c t", t=2)[:, :, 0]
        slots.append((a, bs))
        # k0 (0,3) odd: a = col B+1+2c, b = col B+4+2c
        a = te[:, i, BASE + 1:BASE + 1 + W]
        a = a.rearrange("p (c t) -> p c t", t=2)[:, :, 0]
        bs = te[:, i, BASE + 4:BASE + 4 + W]
        bs = bs.rearrange("p (c t) -> p c t", t=2)[:, :, 0]
        slots.append((a, bs))
    # img2's k0 coverage that the (2-img) DVE k0 op no longer provides:
    # (0,2) full width as two contiguous slots, (0,3) even, (0,1) odd
    for c0 in (0, 512):
        slots.append((
            te[:, 2, BASE + c0:BASE + c0 + 512],
            te[:, 2, BASE + 2 + c0:BASE + 2 + c0 + 512],
        ))
    a = te[:, 2, BASE:BASE + W]
    a = a.rearrange("p (c t) -> p c t", t=2)[:, :, 0]
    bs = te[:, 2, BASE + 3:BASE + 3 + W]
    bs = bs.rearrange("p (c t) -> p c t", t=2)[:, :, 0]
    slots.append((a, bs))
    a = te[:, 2, BASE + 1:BASE + 1 + W]
    a = a.rearrange("p (c t) -> p c t", t=2)[:, :, 0]
    bs = te[:, 2, BASE + 2:BASE + 2 + W]
    bs = bs.rearrange("p (c t) -> p c t", t=2)[:, :, 0]
    slots.append((a, bs))
    return slots


def _build_nc():
    ops = _get_ops()
    nc = bacc_mod.Bacc("TRN2", target_bir_lowering=False)
    # host layout: flat rows 0..H-1; row q = [img0|img1|img2] each WP wide
    # (circular col pads baked in). No row halo: t123 is built on-chip.
    X = nc.dram_tensor(
        "x", [NBLK * ROWS_BLK * ROW + 8], BF16, kind="ExternalInput"
    )
    WI = nc.dram_tensor("wi", [128, 128], BF16, kind="ExternalInput")
    WNI = nc.dram_tensor("wni", [128, 128], BF16, kind="ExternalInput")
    OUT = nc.dram_tensor("out", [128, STAGE_COLS], F32, kind="ExternalOutput")

    with TileContext(nc) as tc:
        with (
            tc.tile_pool(name="te", bufs=2) as te_pool,
            tc.tile_pool(name="t123", bufs=2) as t123_pool,
            tc.tile_pool(name="sc", bufs=1) as sc_pool,
            tc.tile_pool(name="acc", bufs=1) as acc_pool,
            tc.psum_pool(name="ps", bufs=2) as ps_pool,
        ):
            stage = acc_pool.tile([128, STAGE_COLS], F32)
            scratch = sc_pool.tile([128, 3 * ROW], BF16)
            ascr = acc_pool.tile([128, 2], BF16)
            wi = acc_pool.tile([128, 128], BF16)
            wni = acc_pool.tile([128, 128], BF16)
            for r in range(NBLK):
                te_prev = te_pool.tile([128, IMGS, WP], BF16, tag="te")
                t123 = t123_pool.tile([128, T123W], BF16, tag="t123")
                # Blocks 0-1 FIFO on the sync ring (full HBM rate for the
                # pipeline fill); later blocks' big prefetch via GPSIMD's
                # SWDGE so the busy ACT engine issues no DMA. Block 0's
                # t123 is split into per-j section loads so the j=1 trio
                # starts before sections 2-3 land.
                off = (r * ROWS_BLK + 1) * ROW - 2
                nc.sync.dma_start(
                    out=te_prev[:],
                    in_=bass.AP(X, r * ROWS_BLK * ROW,
                                [[ROW, 128], [1, ROW]]),
                )
                if r == 0:
                    nc.sync.dma_start(
                        out=t123[:, 0:ROW + 4],
                        in_=bass.AP(X, off, [[ROW, 128], [1, ROW + 4]]),
                    )
                    nc.sync.dma_start(
                        out=t123[:, ROW + 4:2 * ROW + 4],
                        in_=bass.AP(X, off + ROW + 4, [[ROW, 128], [1, ROW]]),
                    )
                    nc.sync.dma_start(
                        out=t123[:, 2 * ROW + 4:3 * ROW + 4],
                        in_=bass.AP(X, off + 2 * ROW + 4,
                                    [[ROW, 128], [1, ROW]]),
                    )
                    # small constant loads on the idle SWDGE queue so they
                    # don't wait behind the 4MB of block-0 loads
                    nc.gpsimd.dma_start(out=wi[:], in_=WI[:])
                    nc.gpsimd.dma_start(out=wni[:], in_=WNI[:])
                    # pre-load the ACT Abs table before any DVE critical
                    # section (the lazy table-load DMA deadlocks against
                    # critical branches)
                    nc.scalar.activation(
                        out=ascr[:, 0:2],
                        in_=wi[:, 0:2],
                        func=mybir.ActivationFunctionType.Abs,
                    )
                else:
                    eng_b = nc.sync if r == 1 else nc.gpsimd
                    eng_b.dma_start(
                        out=t123[:],
                        in_=bass.AP(X, off, [[ROW, 128], [1, T123W]]),
                    )
                prev_f = te_prev[:].rearrange("p a b -> p (a b)")

                # --- DVE chain: k0 (te only, runs during t123 load),
                # then the j-merged trios (block 0: per-j ops so each is
                # gated only on its own t123 section).
                kind = "seed" if r == 0 else "cont"
                nc.vector._custom_dve(
                    ops[kind],
                    out=scratch[:, 0: 2 * W],
                    in0=te_prev[:, 0:2, BASE:BASE + W],
                    in1=te_prev[:, 0:2, BASE + 2:BASE + 2 + W],
                ).ins.perf_max = 1
                if r == 0:
                    # j=1 ops gated on section 1 only; j=2,3 merged (their
                    # sections land while the j=1 ops run)
                    for d in TRIO_D:
                        nc.vector._custom_dve(
                            ops["cont"],
                            out=scratch[:, 0:ROW],
                            in0=prev_f,
                            in1=t123[:, d + 2:d + 2 + ROW],
                        ).ins.perf_max = 1
                    in0b2 = prev_f.rearrange("p (x c) -> p x c", x=1)
                    in0b2 = in0b2.broadcast_to((128, 2, ROW))
                    for d in TRIO_D:
                        m0 = d + 2 + ROW
                        in1b2 = t123[:, m0:m0 + 2 * ROW].rearrange(
                            "p (j c) -> p j c", j=2
                        )
                        nc.vector._custom_dve(
                            ops["cont"],
                            out=scratch[:, 0:2 * ROW],
                            in0=in0b2,
                            in1=in1b2,
                        ).ins.perf_max = 1
                else:
                    in0b = prev_f.rearrange("p (x c) -> p x c", x=1)
                    in0b = in0b.broadcast_to((128, 3, ROW))
                    for d in TRIO_D:
                        m0 = d + 2
                        in1b = t123[:, m0:m0 + 3 * ROW].rearrange(
                            "p (j c) -> p j c", j=3
                        )
                        nc.vector._custom_dve(
                            ops["cont"],
                            out=scratch[:],
                            in0=in0b,
                            in1=in1b,
                        ).ins.perf_max = 1

                # --- PE + ACT: 24 strided slots in 6 groups of 4
                MMW = 512
                slots = _pe_slots(te_prev, t123)
                for g in range(0, len(slots), 4):
                    grp = slots[g:g + 4]
                    psum = ps_pool.tile([128, 4 * MMW], F32, tag="ps")
                    for m, (brhs, srhs) in enumerate(grp):
                        nc.tensor.matmul(
                            out=psum[:, m * MMW:(m + 1) * MMW],
                            lhsT=wi[:],
                            rhs=brhs,
                            start=True,
                            stop=False,
                        )
                    for m, (brhs, srhs) in enumerate(grp):
                        nc.tensor.matmul(
                            out=psum[:, m * MMW:(m + 1) * MMW],
                            lhsT=wni[:],
                            rhs=srhs,
                            start=False,
                            stop=True,
                        )
                    col = 1 + r * NGRP + g // 4
                    nc.scalar.activation(
                        out=psum[:, 0: len(grp) * MMW],
                        in_=psum[:, 0: len(grp) * MMW],
                        func=mybir.ActivationFunctionType.Abs,
                        accum_out=stage[:, col:col + 1],
                    )
            # ship the ACT columns while the last trio still runs; only
            # col 0 (the DVE accumulator) waits for the read op
            nc.sync.dma_start(
                out=bass.AP(OUT, 1, [[STAGE_COLS, 128], [1, STAGE_COLS - 1]]),
                in_=stage[:, 1:STAGE_COLS],
            )
            nc.vector._custom_dve(
                ops["read"],
                out=stage[:, 0:1],
                in0=scratch[:, 0:1],
            )
            nc.sync.dma_start(
                out=bass.AP(OUT, 0, [[STAGE_COLS, 128], [1, 1]]),
                in_=stage[:, 0:1],
            )
    return nc


_NC = None


def _get_nc():
    global _NC
    if _NC is None:
        _NC = _build_nc()
        if not _NC.is_finalized():
            _NC.finalize()
    return _NC


def _prep_shards(x: np.ndarray):
    """bf16-cast, circular col pad, flatten rows into per-core layout."""
    imgs = np.ascontiguousarray(x.reshape(B * C, H, W), dtype=np.float32)

    def to_bf16(a32):
        b = a32.view(np.uint32)
        return ((b + 0x7FFF + ((b >> 16) & 1)) >> 16).astype(np.uint16)

    imgs_b = to_bf16(imgs)  # (24, H, W) uint16 view of bf16
    HPAD = H + 3
    even = np.zeros((B * C, HPAD, WP), dtype=np.uint16)
    even[:, :H, BASE:BASE + W] = imgs_b
    even[:, :H, :BASE] = imgs_b[:, :, W - BASE:]
    even[:, :H, BASE + W:BASE + W + 3] = imgs_b[:, :, :3]
    even[:, H:, :] = even[:, :3, :]

    I = np.eye(128, dtype=np.float32)
    wi = to_bf16(I)
    wni = to_bf16(-I)

    shards = even.reshape(NCORES, IMGS, HPAD, WP)
    out = []
    pcorr = []
    for n in range(NCORES):
        t = shards[n].transpose(1, 0, 2)  # (HPAD, IMGS, WP)
        blk = np.empty((NBLK, ROWS_BLK, IMGS, WP), dtype=np.uint16)
        for r in range(NBLK):
            blk[r] = t[r * RB: r * RB + ROWS_BLK]
        flat = np.concatenate([blk.reshape(-1), np.zeros(8, np.uint16)])
        out.append({"x": flat, "wi": wi, "wni": wni})
        # Exact correction for the trio ops' pad-column junk terms: the
        # 4 pad a-pairs per img per row contribute F+E+O terms with
        # b read at flat offset +j*ROW+d, exactly as the device t123
        # tile is laid out.
        af = (flat.astype(np.uint32) << 16).view(np.float32).astype(np.float64)
        rowbase = (
            (ROWS_BLK * np.arange(NBLK)[:, None] + np.arange(128)[None, :])
            * ROW
        )
        P = 0.0
        for j in (1, 2, 3):
            for d in TRIO_D:
                for i in range(IMGS):
                    for c in (i * WP + 0, i * WP + 2,
                              i * WP + BASE + W, i * WP + BASE + W + 2):
                        ai = rowbase + c
                        bi = ai + j * ROW + d
                        alo, ahi = af[ai], af[ai + 1]
                        blo, bhi = af[bi], af[bi + 1]
                        P += (np.abs(alo - blo) + np.abs(ahi - bhi)
                              + np.abs(alo - bhi) + np.abs(ahi - blo)).sum()
        pcorr.append(P)
    return out, pcorr


def _run(x: np.ndarray, trace: bool = False):
    import ml_dtypes

    nc = _get_nc()
    in_maps, pcorr = _prep_shards(x)
    in_maps = [
        {k: v.view(ml_dtypes.bfloat16) for k, v in m.items()} for m in in_maps
    ]
    res = run_bass_kernel_spmd(
        nc, in_maps, core_ids=list(range(NCORES)), trace=trace
    )
    total = 0.0
    for r, pc in zip(res.results, pcorr):
        total += r["out"].astype(np.float64).sum() - pc
    val = WEIGHT * 2.0 * total / float(B * C * H * W)
    return np.float32(val), res


def kernel(x: np.ndarray) -> np.ndarray:
    x = np.asarray(x, dtype=np.float32)
    val, _ = _run(x, trace=False)
    return val
